# revision 16
# baseline (speedup 1.0000x reference)
"""CMFM loss kernel for Trainium2 (8 NeuronCores, Bass/Tile) — v4.

Math: for inputs f_v, f_a [B,T,D] with vn/an the D-normalized tensors,
  cos[b,t]    = s_va / (sqrt(s_vv)*sqrt(s_aa))          (per-timestep term)
  sum_{i!=j} cross = (1/T)*(sum_t V_t.A_t  -  sum_{b,t} cos[b,t])
where V_t = sum_b vn[b,t,:], A_t = sum_b an[b,t,:].  Data-parallel over B
(8 rows/core); host combines the 8 partial V/A tensors and cos stats.

Design (HW-microbenchmarked per-op costs in ns for [128,256] fp16):
  * fp16 device data (host cast): halves DMA-in (8.4MB/core); DMA layout
    packs two consecutive t per partition (t = j*256 + 2p + k) so every
    DMA descriptor moves 1KB contiguous.
  * Reductions: HW benches show scalar_tensor_tensor is 1x (378),
    ACT Square+accum 650, but tensor_tensor fp16 is 2x (224) and grouped
    tensor_reduce all-fp16 is ~4x (153/256elems).  So squares and s_va
    run as row-batched TT-multiply [128,2048] + grouped tensor_reduce
    -> [128,8] fp16 = ~223ns/tile on DVE; a balance knob K_ZACT moves
    some row-kinds to ACT per-tile Square+accum (650) to equalize
    engines (~42us busy each at z=7 of 16).
  * Normalize-accumulate rides the otherwise idle PE: stationary =
    diag(1/|x|) as four 32x32 diagonal blocks via tile_position
    (HW: 146ns per 4-block group, concurrent sub-arrays), accumulated
    across the 8 batch rows in PSUM f32.  Diag matrices built on DVE with
    one broadcast-STT per row (block-diag cols, 32-wide).
  * Identity comes from HBM (no GPSIMD anywhere -> cheap end-of-block
    drain); ACT table preloaded via a dummy Square during the first DMA;
    V/A leave PSUM via per-half-row fp16 casts (ACT for v, DVE for a)
    then DMA; the last row runs ACT-heavy and with half-row s_va chunks
    to shorten the tail chain sqrt->recip->diag->MM->copy->DMA.

Known stack quirks: InstTensorTensorReduce crashes the NRT; Pool rejects
TensorScalarPtr; Pool TT offload measures far worse than modeled; ACT
Rsqrt/Reciprocal banned -> Sqrt + DVE reciprocal; DMA cannot read PSUM.
"""

import os as _os

import numpy as np

import concourse.bacc as bacc
import concourse.bass as bass
import concourse.tile as tile
from concourse import mybir
from concourse.bass_utils import run_bass_kernel_spmd

ALPHA, BETA, GAMMA = 2.0, 2.0, 1.0
B, T, D = 64, 1024, 256
N_CORES = 8
B_LOC = B // N_CORES          # 8 batch rows per core
P = 128                       # SBUF partitions
KP = 2                        # t-pack: consecutive t per partition
JCH = T // (P * KP)           # 4 j-chunks per row
TCH = JCH * KP                # 8 (j,k) tiles per row; t = j*256 + 2p + k
NT = B_LOC * TCH              # 64 cos-stat columns per core

F32 = mybir.dt.float32
F16 = mybir.dt.float16
MULT = mybir.AluOpType.mult
ADD = mybir.AluOpType.add

# row-kinds (of 16 = 8 rows x {v,a}) whose squares run per-tile on ACT;
# the rest run row-batched TT+reduce on DVE.  Row 7 is always on ACT so
# the tail sqrt isn't gated on a long DVE queue.
K_ZACT = int(_os.environ.get("K_ZACT", "7"))
# PE accumulate variant: "block" = 4x 32x32 diag blocks, "full" = 128x128
K_PE = _os.environ.get("K_PE", "block")
K_IO_BUFS = int(_os.environ.get("K_IO_BUFS", "6"))

_CACHE = {}
LAST_RESULTS = None


def _act_rows():
    """Pick which of the 14 non-last row-kinds use ACT squares."""
    z = max(0, min(14, K_ZACT - 2))
    picks = set()
    i = 0
    while len(picks) < z:
        picks.add((i * 5) % 14)
        i += 1
        if i > 40:
            picks.update(range(z))
            break
    return picks


def _build_nc(repeat=1, loop_n=1):
    """Per-core Bass program.  repeat/loop_n re-run the pass for slope timing."""
    nc = bacc.Bacc("TRN2", debug=False)

    DGW = 32 if K_PE == "block" else P
    act_rows = _act_rows()
    v = nc.dram_tensor("v", [B_LOC, T, D], F16, kind="ExternalInput").ap()
    a = nc.dram_tensor("a", [B_LOC, T, D], F16, kind="ExternalInput").ap()
    ident = nc.dram_tensor("ident", [P, DGW], F16, kind="ExternalInput").ap()
    cos_out = nc.dram_tensor("cos_stat", [P, NT], F32, kind="ExternalOutput").ap()
    vacc_out = nc.dram_tensor("v_acc", [P, TCH * D], F16, kind="ExternalOutput").ap()
    aacc_out = nc.dram_tensor("a_acc", [P, TCH * D], F16, kind="ExternalOutput").ap()

    with tile.TileContext(nc) as tc:
        with (
            tc.tile_pool(name="io", bufs=K_IO_BUFS) as io_pool,
            tc.tile_pool(name="big", bufs=4) as bigp,
            tc.tile_pool(name="scratch", bufs=8) as scratch,
            tc.tile_pool(name="small", bufs=4) as small,
            tc.tile_pool(name="diagp", bufs=2) as diagp,
            tc.tile_pool(name="acc", bufs=1) as accp,
            tc.tile_pool(name="psum", bufs=1, space="PSUM") as psump,
        ):
            nrm2 = accp.tile([P, B_LOC, TCH, 2], F32)   # (s_vv, s_aa)
            inv_stat = accp.tile([P, B_LOC, TCH, 2], F32)
            sva16 = accp.tile([P, B_LOC, TCH], F16)
            cos_stat = accp.tile([P, NT], F32)
            ii = accp.tile([P, NT], F32)

            ps_v = psump.tile([P, TCH, D], F32)         # V partial, 4 banks
            ps_a = psump.tile([P, TCH, D], F32)         # A partial, 4 banks

            id_blk = accp.tile([P, DGW], F16)
            nc.sync.dma_start(out=id_blk[:], in_=ident[:, :])
            warm = accp.tile([P, 1], F16)
            nc.scalar.activation(out=warm[:], in_=id_blk[:, 0:1],
                                 func=mybir.ActivationFunctionType.Square)

            import contextlib
            loop_ctx = (
                tc.For_i(
                    0, loop_n, 1,
                    hint_engines=(
                        mybir.EngineType.DVE,
                        mybir.EngineType.Activation,
                        mybir.EngineType.PE,
                        mybir.EngineType.SP,
                    ),
                )
                if loop_n > 1
                else contextlib.nullcontext()
            )

            HREST = TCH // 2  # stage2 half-row granularity

            def sq_reduce_dve(sup, b, kind, nchunk=1):
                """Row-kind squares: TT sup*sup then grouped reduce to
                [P,TCH] fp16, cast into the f32 nrm2 slice."""
                for c in range(nchunk):
                    jw = JCH // nchunk
                    js = slice(c * jw, (c + 1) * jw)
                    sq = bigp.tile([P, JCH, KP * D], F16, tag=f"sq{kind}")
                    nc.vector.tensor_tensor(
                        out=sq[:, js], in0=sup[:, js], in1=sup[:, js], op=MULT)
                    r16 = small.tile([P, TCH], F16, tag=f"r{kind}")
                    with nc.allow_low_precision(reason="norm^2 in fp16 (~1e-3)"):
                        nc.vector.tensor_reduce(
                            out=r16[:, c * jw * KP:(c + 1) * jw * KP],
                            in_=sq[:, js].rearrange("p j (k d) -> p (j k) d", k=KP),
                            axis=mybir.AxisListType.X, op=ADD)
                    nc.vector.tensor_copy(
                        out=nrm2[:, b, c * jw * KP:(c + 1) * jw * KP, kind],
                        in_=r16[:, c * jw * KP:(c + 1) * jw * KP])

            def sva_reduce(v_s, a_s, b, nchunk=1):
                for c in range(nchunk):
                    jw = JCH // nchunk
                    js = slice(c * jw, (c + 1) * jw)
                    prod = bigp.tile([P, JCH, KP * D], F16, tag="prod")
                    nc.vector.tensor_tensor(
                        out=prod[:, js], in0=v_s[:, js], in1=a_s[:, js], op=MULT)
                    with nc.allow_low_precision(reason="s_va in fp16 (~5e-4)"):
                        nc.vector.tensor_reduce(
                            out=sva16[:, b, c * jw * KP:(c + 1) * jw * KP],
                            in_=prod[:, js].rearrange("p j (k d) -> p (j k) d", k=KP),
                            axis=mybir.AxisListType.X, op=ADD)

            def emit_stage2(pend, half):
                """sqrt/recip + diag build (whole row at half 0) + PE
                accumulate for half a row; last row streams V/A out."""
                b, v_s, a_s, dgs = pend
                t0 = half * HREST
                if half == 0:
                    nsq = small.tile([P, TCH, 2], F32, tag="nsq")
                    nc.scalar.activation(out=nsq[:], in_=nrm2[:, b],
                                         func=mybir.ActivationFunctionType.Sqrt)
                    nc.vector.reciprocal(out=inv_stat[:, b], in_=nsq[:])
                    # block-diag: dg[p,(tc,kind),c] = id[p,c]*inv[p,tc,kind]
                    nc.vector.scalar_tensor_tensor(
                        out=dgs[:], scalar=1.0, op0=MULT, op1=MULT,
                        in0=id_blk[:].unsqueeze(1).broadcast_to((P, TCH * 2, DGW)),
                        in1=inv_stat[:, b].rearrange("p t k -> p (t k)")
                            .unsqueeze(2).broadcast_to((P, TCH * 2, DGW)),
                    )
                for tci in range(t0, t0 + HREST):
                    j, k = tci // KP, tci % KP
                    for kind, (sup, ps) in enumerate(
                            ((v_s, ps_v), (a_s, ps_a))):
                        mov = sup[:, j, k * D:(k + 1) * D]
                        dcol = tci * 2 + kind
                        if K_PE == "block":
                            for i in range(4):
                                sl = slice(32 * i, 32 * (i + 1))
                                nc.tensor.matmul(
                                    out=ps[sl, tci, :],
                                    lhsT=dgs[sl, dcol, :],
                                    rhs=mov[sl],
                                    start=(b == 0), stop=(b == B_LOC - 1),
                                    tile_position=(32 * i, 32 * i),
                                    skip_group_check=True,
                                )
                        else:
                            nc.tensor.matmul(
                                out=ps[:, tci, :],
                                lhsT=dgs[:, dcol, :],
                                rhs=mov,
                                start=(b == 0), stop=(b == B_LOC - 1),
                                skip_group_check=True,
                            )
                if b == B_LOC - 1:
                    # evacuate the finished half-row PSUM regions: fp16
                    # casts, v on ACT and a on DVE in parallel, then DMA.
                    lo = t0 * D
                    for kind, (ps, dst) in enumerate(
                            ((ps_v, vacc_out), (ps_a, aacc_out))):
                        sb = scratch.tile([P, HREST, D], F16, tag="acco")
                        if kind == 0:
                            nc.scalar.activation(
                                out=sb[:], in_=ps[:, t0:t0 + HREST, :],
                                func=mybir.ActivationFunctionType.Copy)
                        else:
                            nc.vector.tensor_copy(
                                out=sb[:], in_=ps[:, t0:t0 + HREST, :])
                        nc.sync.dma_start(out=dst[:, lo:lo + HREST * D],
                                          in_=sb[:])

            with loop_ctx:
              for _ in range(repeat):
                pend = None
                for b in range(B_LOC):
                    last = b == B_LOC - 1
                    v_s = io_pool.tile([P, JCH, KP * D], F16, tag="vt")
                    a_s = io_pool.tile([P, JCH, KP * D], F16, tag="at")
                    vr = v[b].rearrange("(j p k) d -> p j (k d)", p=P, k=KP)
                    ar = a[b].rearrange("(j p k) d -> p j (k d)", p=P, k=KP)
                    nspl = 2 if b == 0 else 1
                    hh = JCH // nspl
                    for s_ in range(nspl):
                        nc.sync.dma_start(
                            out=v_s[:, s_ * hh:(s_ + 1) * hh, :],
                            in_=vr[:, s_ * hh:(s_ + 1) * hh, :])
                        nc.sync.dma_start(
                            out=a_s[:, s_ * hh:(s_ + 1) * hh, :],
                            in_=ar[:, s_ * hh:(s_ + 1) * hh, :])

                    # squares for this row's two kinds
                    for kind, sup in ((0, v_s), (1, a_s)):
                        if last or (b * 2 + kind) in act_rows:
                            for tci in range(TCH):
                                j, k = tci // KP, tci % KP
                                t_ = sup[:, j, k * D:(k + 1) * D]
                                sq = scratch.tile([P, D], F16, tag="sqa")
                                nc.scalar.activation(
                                    out=sq[:], in_=t_,
                                    func=mybir.ActivationFunctionType.Square,
                                    accum_out=nrm2[:, b, tci, kind:kind + 1])
                        else:
                            sq_reduce_dve(sup, b, kind)
                    # s_va (always DVE); half-row chunks on the last row
                    sva_reduce(v_s, a_s, b, nchunk=2 if last else 1)

                    if pend is not None:
                        emit_stage2(pend, 0)
                        emit_stage2(pend, 1)
                    dgs = diagp.tile([P, TCH * 2, DGW], F16, tag="dg")
                    pend = (b, v_s, a_s, dgs)
                emit_stage2(pend, 0)
                emit_stage2(pend, 1)

                # cos = s_va * inv_v * inv_a
                nc.vector.tensor_mul(
                    out=ii[:],
                    in0=inv_stat[:].rearrange("p b t k -> p (b t) k")[:, :, 0],
                    in1=inv_stat[:].rearrange("p b t k -> p (b t) k")[:, :, 1])
                nc.vector.tensor_mul(
                    out=cos_stat[:], in0=ii[:],
                    in1=sva16[:].rearrange("p b t -> p (b t)"))
                nc.sync.dma_start(out=cos_out[:, :], in_=cos_stat[:])

    nc.compile()
    return nc


def _get_nc(repeat=1, loop_n=1):
    key = ("nc", repeat, loop_n, K_ZACT, K_PE)
    if key not in _CACHE:
        _CACHE[key] = _build_nc(repeat, loop_n)
    return _CACHE[key]


def _ident_np():
    dgw = 32 if K_PE == "block" else P
    e = np.zeros((P, dgw), np.float16)
    for p in range(P):
        e[p, p % dgw] = 1.0
    return e


def _run(nc, f_v16, f_a16):
    ident = _ident_np()
    in_maps = [
        {
            "v": np.ascontiguousarray(f_v16[c * B_LOC:(c + 1) * B_LOC]),
            "a": np.ascontiguousarray(f_a16[c * B_LOC:(c + 1) * B_LOC]),
            "ident": ident,
        }
        for c in range(N_CORES)
    ]
    return run_bass_kernel_spmd(nc, in_maps, core_ids=list(range(N_CORES)))


def kernel(f_v, f_a, labels):
    global LAST_RESULTS
    f_v16 = np.asarray(f_v, dtype=np.float32).astype(np.float16)
    f_a16 = np.asarray(f_a, dtype=np.float32).astype(np.float16)
    labels = np.asarray(labels)

    res = _run(_get_nc(), f_v16, f_a16)
    LAST_RESULTS = res
    out = res.results

    # layout-free reductions: row_cos[b] sums that row's 8 stat columns;
    # cross needs only the elementwise product of identically-laid-out V/A.
    cos = np.stack([out[c]["cos_stat"] for c in range(N_CORES)])  # [C,P,NT]
    cos = cos.reshape(N_CORES, P, B_LOC, TCH)
    row_cos = cos.sum(axis=(1, 3), dtype=np.float64).reshape(B)

    v_acc = np.zeros((P, TCH * D), np.float64)
    a_acc = np.zeros((P, TCH * D), np.float64)
    for c in range(N_CORES):
        v_acc += out[c]["v_acc"]
        a_acc += out[c]["a_acc"]
    cross_sum = float((v_acc * a_acc).sum())   # = sum_t V_t . A_t

    pos = labels == 0
    n_pos = int(pos.sum())
    n_neg = B - n_pos

    loss_pos = ALPHA * (n_pos * T - row_cos[pos].sum())
    loss_neg = BETA * row_cos[~pos].sum()
    loss_neg += GAMMA * (cross_sum - row_cos.sum()) / T
    cnt_pos = n_pos * T
    cnt_neg = n_neg * T + B * (B - 1)

    loss = 0.0
    if cnt_pos > 0:
        loss += loss_pos / max(cnt_pos, 1.0)
    if cnt_neg > 0:
        loss += loss_neg / max(cnt_neg, 1.0)
    return np.float32(loss)
